# revision 15
# baseline (speedup 1.0000x reference)
"""BitTransformerBlock on 8 Trainium2 NeuronCores.

Token-parallel sharding: the flattened (B*S)=4096 tokens are split 512 per
core; cores 0-3 hold batch 0, cores 4-7 batch 1.  Each core computes LN1 and
the q/k/v projections for its own tokens; four small in-kernel AllGathers
(k-lo, v-lo, k-hi, v-hi; replica groups [0..3], [4..7]) share K and V across
each batch group pipelined against the projections, and everything
downstream (attention over the full 2048-token context, out-proj, LN2, the
FFN) is token-local.

Numerics: most matmuls run fp8(e4m3) with DoubleRow (2 contraction rows per
PE cell, half the matmul instructions); scores stay bf16.  The BitNet
act_quant round-trips are skipped entirely (quantization noise is ~1e-3 on
the final output); the host-ternarized weights {-1,0,1} are fp8-exact and
their scales s1/s2 fold into the gelu scale and the final residual fma.
Softmax runs without max subtraction (logits are small); exp is split
between the Scalar engine (table exp -> fp8) and the Vector engine
(Schraudolph fast-exp: fma to int8, bitcast to fp8e4m3).  All transposes
are PE transposes through PSUM.  Softmax denominators are batched per
half: DRAM bounce, partition-broadcast DMA, fast reciprocal, one multiply.
"""

import numpy as np
import ml_dtypes

import concourse.bacc as bacc
import concourse.bass as bass
import concourse.mybir as mybir
import concourse.tile as tile
from concourse.bass_interp import get_hw_module
from concourse.bass_utils import run_bass_kernel_spmd

F32 = mybir.dt.float32
BF16 = mybir.dt.bfloat16
F8 = mybir.dt.float8e4
I16 = mybir.dt.int16
I8 = mybir.dt.int8
AF = mybir.ActivationFunctionType
OP = mybir.AluOpType
DR = mybir.MatmulPerfMode.DoubleRow

N_CORES = 8
B, S, D, H, FF = 2, 2048, 1024, 16, 4096
HD = D // H                 # 64
NTOK = B * S                # 4096
TOK = NTOK // N_CORES       # 512 tokens per core
TCH = TOK // 128            # 4 token chunks per core
DCH = D // 128              # 8
FFCH = FF // 128            # 32
NKC = S // 128              # 16 key chunks per batch
GROUPS = [[0, 1, 2, 3], [4, 5, 6, 7]]
CORES_PER_B = 4
EPS = 1e-5
INV_SQRT_HD = 1.0 / 8.0
# Schraudolph fast-exp in fp8e4m3: bits8 = round(x * 2^3/ln2 + (7-c)*2^3)
FEXP_A = float(2.0 ** 3 / np.log(2.0))
FEXP_B = float((7.0 - 0.043) * 2 ** 3)


def _bcast_part(ap, parts):
    """View a [1, F] (or [F]) AP as [parts, F] via a zero-stride partition dim."""
    inner = [list(e) for e in ap.ap if e[1] != 1] or [[1, 1]]
    return bass.AP(tensor=ap.tensor, offset=ap.offset, ap=[[0, parts]] + inner)


def build_program(s1, s2, biases, sim_gelu=False):
    nc = bacc.Bacc("TRN2", target_bir_lowering=False, debug=False,
                   num_devices=N_CORES)

    x_in = nc.dram_tensor("x_sh", [TOK, D], F32, kind="ExternalInput")
    wq_in = nc.dram_tensor("wqT", [D, D], F8, kind="ExternalInput")
    wk_in = nc.dram_tensor("wkT", [D, D], F8, kind="ExternalInput")
    wv_in = nc.dram_tensor("wvT", [D, D], F8, kind="ExternalInput")
    wo_in = nc.dram_tensor("woT", [D, D], F8, kind="ExternalInput")
    w1_in = nc.dram_tensor("w1T", [D, FF], F8, kind="ExternalInput")
    w2_in = nc.dram_tensor("w2T", [FF, D], F8, kind="ExternalInput")
    id_in = nc.dram_tensor("ident", [128, 128], BF16, kind="ExternalInput")
    out_d = nc.dram_tensor("out", [TOK, D], F32, kind="ExternalOutput")

    ext = {}
    if biases["ln1_g"]:
        ext["ln1_g"] = nc.dram_tensor("ln1_g", [D], F32, kind="ExternalInput")
    if biases["ln1_b"]:
        ext["ln1_b"] = nc.dram_tensor("ln1_b", [D], F32, kind="ExternalInput")
    if biases["ln2_g"]:
        ext["ln2_g"] = nc.dram_tensor("ln2_g", [D], F32, kind="ExternalInput")
    if biases["ln2_b"]:
        ext["ln2_b"] = nc.dram_tensor("ln2_b", [D], F32, kind="ExternalInput")
    if biases["in_proj_b"]:
        ext["in_b"] = nc.dram_tensor("in_b", [3 * D], F32, kind="ExternalInput")
    if biases["out_proj_b"]:
        ext["out_b"] = nc.dram_tensor("out_b", [D], F32, kind="ExternalInput")
    if biases["b1"]:
        ext["b1"] = nc.dram_tensor("b1", [FF], F32, kind="ExternalInput")
    if biases["b2"]:
        ext["b2"] = nc.dram_tensor("b2", [D], F32, kind="ExternalInput")

    with tile.TileContext(nc) as tc:
        _emit(nc, tc, x_in, wq_in, wk_in, wv_in, wo_in, w1_in, w2_in, id_in,
              out_d, ext, s1, s2, biases, sim_gelu)
    nc.compile()
    return nc


def _emit(nc, tc, x_in, wq_in, wk_in, wv_in, wo_in, w1_in, w2_in, id_in,
          out_d, ext, s1, s2, biases, sim_gelu=False):
    gelu_func = AF.Tanh if sim_gelu else AF.Gelu
    from contextlib import ExitStack

    def scope(name):
        sid = nc.enter_named_scope(name, False)
        return (name, sid[0] if isinstance(sid, tuple) else sid)

    def unscope(tok):
        nc.leave_named_scope(tok[0], tok[1], False)

    es_top = ExitStack()
    dram = es_top.enter_context(tc.tile_pool(name="dram", bufs=1, space="DRAM"))
    const = es_top.enter_context(tc.tile_pool(name="const", bufs=1))
    stats = es_top.enter_context(tc.tile_pool(name="stats", bufs=4))

    kv_bnc = [dram.tile([2 * (D // 2), TOK], F8, name=f"kv_bnc{i}")
              for i in range(2)]
    kv_all = [dram.tile([CORES_PER_B * 2 * (D // 2), TOK], F8,
                        name=f"kv_all{i}") for i in range(2)]
    den_dram = dram.tile([H, TOK], BF16)

    eps_t = const.tile([128, 1], F32)
    nc.vector.memset(eps_t[:], EPS)
    ident = const.tile([128, 128], BF16, tag="ident")
    nc.sync.dma_start(out=ident[:], in_=id_in[:])

    def load_bcast(name, width, src_ap):
        t = const.tile([128, width], F32, tag=f"bc_{name}")
        nc.sync.dma_start(out=t[:], in_=_bcast_part(src_ap, 128))
        return t

    g1_bc = load_bcast("g1", D, ext["ln1_g"][:]) if biases["ln1_g"] else None
    b1ln_bc = load_bcast("b1ln", D, ext["ln1_b"][:]) if biases["ln1_b"] else None
    g2_bc = load_bcast("g2", D, ext["ln2_g"][:]) if biases["ln2_g"] else None
    b2ln_bc = load_bcast("b2ln", D, ext["ln2_b"][:]) if biases["ln2_b"] else None
    bv_bc = (load_bcast("bv", D, ext["in_b"][2 * D:3 * D])
             if biases["in_proj_b"] else None)
    bo_bc = load_bcast("bo", D, ext["out_b"][:]) if biases["out_proj_b"] else None
    bf2_bc = load_bcast("bf2", D, ext["b2"][:]) if biases["b2"] else None
    bq_fm = bk_fm = None
    if biases["in_proj_b"]:
        bq_fm = const.tile([128, DCH], F32, tag="bq_fm")
        nc.sync.dma_start(out=bq_fm[:], in_=ext["in_b"][0:D].rearrange("(c p) -> p c", p=128))
        bk_fm = const.tile([128, DCH], F32, tag="bk_fm")
        nc.sync.dma_start(out=bk_fm[:], in_=ext["in_b"][D:2 * D].rearrange("(c p) -> p c", p=128))
    if biases["b1"]:
        bf1_fm = const.tile([128, FFCH], F32, tag="bf1_fm")
        nc.sync.dma_start(out=bf1_fm[:], in_=ext["b1"][:].rearrange("(c p) -> p c", p=128))

    # ---- pool stack (lifetimes nest: later-opened closes first) ----------
    es_D = ExitStack()
    pD = es_D.enter_context(tc.tile_pool(name="pD", bufs=1))     # x2, w1
    es_E = ExitStack()
    pE = es_E.enter_context(tc.tile_pool(name="pE", bufs=1))     # hqT
    es_C = ExitStack()
    pC = es_C.enter_context(tc.tile_pool(name="pC", bufs=1))     # oun/den/wo/oT
    es_B = ExitStack()
    pB = es_B.enter_context(tc.tile_pool(name="pB", bufs=1))     # KT/Vaug/qT
    es_X = ExitStack()
    pX = es_X.enter_context(tc.tile_pool(name="pX", bufs=1))     # nxT

    x2 = pD.tile([128, TCH, D], F32, tag="x2")
    w1_sb = pD.tile([128, DCH, FF], F8, tag="w1")
    hqT = pE.tile([128, DCH, TOK], F8, tag="hqT")
    oun_sb = pC.tile([128, H // 2, TOK], BF16, tag="oun")
    den_sb = pC.tile([1, H, TOK], BF16, tag="den")
    wo_sb = pC.tile([128, DCH, D], F8, tag="wo")
    oT = pC.tile([128, DCH, TOK], F8, tag="oT")
    den_scr = pC.tile([128, H // 8, TOK], BF16, tag="denscr")   # per-quarter
    den_bcf = pC.tile([128, H // 8, TOK], F32, tag="denbcf")
    den_inv = pC.tile([128, H // 8, TOK], F32, tag="deninv")
    KT = pB.tile([128, DCH, CORES_PER_B, 512], F8, tag="KT")
    Vaug = pB.tile([128, NKC, H * (HD + 1)], F8, tag="Va")
    qT_sb = pB.tile([128, DCH, TOK], F8, tag="qT")
    nxT = pX.tile([128, DCH, TOK], F8, tag="nxT")

    def ln_stats(src_tile, tag):
        """All-chunk LN stats: one Ln + one Exp.  Returns (mvs, rstd)."""
        mvs = stats.tile([128, TCH, 2], F32, tag=f"mvs_{tag}", name=f"mvs_{tag}")
        for c in range(TCH):
            st = stats.tile([128, 2, 6], F32, tag="bnst")
            nc.vector.bn_stats(out=st[:, 0, :], in_=src_tile[:, c, 0:512])
            nc.vector.bn_stats(out=st[:, 1, :], in_=src_tile[:, c, 512:1024])
            nc.vector.bn_aggr(out=mvs[:, c, :], in_=st[:])
        rstd = stats.tile([128, TCH], F32, tag=f"rstd_{tag}", name=f"rstd_{tag}")
        var_v = bass.AP(tensor=mvs[:].tensor, offset=mvs[:].offset + 1,
                        ap=[list(mvs[:].ap[0])] + [[2, TCH]])
        nc.scalar.activation(out=rstd[:], in_=var_v, func=AF.Ln, bias=eps_t[:])
        nc.scalar.activation(out=rstd[:], in_=rstd[:], func=AF.Exp, scale=-0.5)
        return mvs, rstd

    def ln_norm_chunk(src_ap, mvs, rstd, c, g_bc, b_bc, out_tile):
        nc.vector.tensor_scalar(out=out_tile, in0=src_ap,
                                scalar1=mvs[:, c, 0:1], scalar2=rstd[:, c:c + 1],
                                op0=OP.subtract, op1=OP.mult)
        if g_bc is not None:
            nc.vector.tensor_mul(out=out_tile, in0=out_tile, in1=g_bc[:])
        if b_bc is not None:
            nc.vector.tensor_add(out=out_tile, in0=out_tile, in1=b_bc[:])

    # ---- stage 1: load x, LN1, PE-transpose to nxT (fp8) -----------------
    sc = scope("ln1")
    es_1 = ExitStack()
    s1p = es_1.enter_context(tc.tile_pool(name="s1p", bufs=1))
    s1n = es_1.enter_context(tc.tile_pool(name="s1n", bufs=3))
    ps_t = es_1.enter_context(tc.tile_pool(name="ps_t", bufs=4, space="PSUM"))
    x_v = x_in.rearrange("(i p) d -> p i d", p=128)
    x_sb = s1p.tile([128, TCH, D], F32, tag="xsb", name="x_sb")
    for i in range(TCH):
        nc.sync.dma_start(out=x_sb[:, i, :], in_=x_v[:, i, :])
    mvs1, rstd1 = ln_stats(x_sb, "ln1")
    for i in range(TCH):
        nxt = s1n.tile([128, D], BF16, tag="nx")
        ln_norm_chunk(x_sb[:, i, :], mvs1, rstd1, i, g1_bc, b1ln_bc, nxt[:])
        for dc in range(DCH):
            pt = ps_t.tile([128, 128], BF16, tag="pt")
            nc.tensor.transpose(pt[:], nxt[:, dc * 128:(dc + 1) * 128], ident[:])
            nc.vector.tensor_copy(out=nxT[:, dc, i * 128:(i + 1) * 128], in_=pt[:])
    es_1.close()
    unscope(sc)

    # ---- stage 2: k/v/q projections + 4 pipelined AllGathers -------------
    sc = scope("inproj")
    es_3 = ExitStack()
    pW = es_3.enter_context(tc.tile_pool(name="pW", bufs=3))
    ps3 = es_3.enter_context(tc.tile_pool(name="ps3", bufs=4, space="PSUM"))
    s3 = es_3.enter_context(tc.tile_pool(name="s3", bufs=1))

    wk_v = wk_in.rearrange("(c p) f -> p c f", p=128)
    wq_v = wq_in.rearrange("(c p) f -> p c f", p=128)
    wv_v = wv_in.rearrange("(c p) f -> p c f", p=128)

    kT_loc = s3.tile([128, DCH, 512], F8, tag="kT_loc", name="kT_loc")
    v_loc = s3.tile([128, TCH, D], F8, tag="v_loc", name="v_loc")

    def kq_proj_half(w_view, half, out_tile, bias_fm):
        wt = pW.tile([128, DCH, 512], F8, tag="wslc")
        nc.sync.dma_start(out=wt[:], in_=w_view[:, :, half * 512:(half + 1) * 512])
        for fi in range(4):
            fo = half * 4 + fi
            ps = ps3.tile([128, 512], F32, tag="ps")
            for d2 in range(DCH // 2):
                nc.tensor.matmul(ps[:],
                                 lhsT=wt[:, 2 * d2:2 * d2 + 2, fi * 128:(fi + 1) * 128],
                                 rhs=nxT[:, 2 * d2:2 * d2 + 2, :],
                                 start=(d2 == 0), stop=(d2 == DCH // 2 - 1),
                                 perf_mode=DR)
            if biases["in_proj_b"]:
                nc.scalar.activation(out=out_tile[:, fo, :], in_=ps[:],
                                     func=AF.Identity, bias=bias_fm[:, fo:fo + 1])
            else:
                nc.scalar.activation(out=out_tile[:, fo, :], in_=ps[:], func=AF.Copy)

    def v_proj_half(f2):
        wt = pW.tile([128, DCH, 512], F8, tag="wslc")
        nc.sync.dma_start(out=wt[:], in_=wv_v[:, :, f2 * 512:(f2 + 1) * 512])
        for to in range(TCH):
            ps = ps3.tile([128, 512], F32, tag="ps")
            for d2 in range(DCH // 2):
                nc.tensor.matmul(ps[:],
                                 lhsT=nxT[:, 2 * d2:2 * d2 + 2, to * 128:(to + 1) * 128],
                                 rhs=wt[:, 2 * d2:2 * d2 + 2, :],
                                 start=(d2 == 0), stop=(d2 == DCH // 2 - 1),
                                 perf_mode=DR)
            dst = v_loc[:, to, f2 * 512:(f2 + 1) * 512]
            if biases["in_proj_b"]:
                nc.vector.tensor_add(out=dst, in0=ps[:],
                                     in1=bv_bc[:, f2 * 512:(f2 + 1) * 512])
            else:
                nc.vector.tensor_copy(out=dst, in_=ps[:])

    for half in range(2):
        kq_proj_half(wk_v, half, kT_loc, bk_fm)
        kv_ap = kv_bnc[half][:]
        kdst = bass.AP(tensor=kv_ap.tensor, offset=kv_ap.offset,
                       ap=[[TOK, 128], [128 * TOK, 4], [1, TOK]])
        nc.sync.dma_start(out=kdst, in_=kT_loc[:, half * 4:(half + 1) * 4, :])
        v_proj_half(half)
        vdst = bass.AP(tensor=kv_ap.tensor, offset=kv_ap.offset + 512 * TOK,
                       ap=[[512, 128], [128 * 512, 4], [1, 512]])
        nc.sync.dma_start(out=vdst, in_=v_loc[:, :, half * 512:(half + 1) * 512])
        nc.gpsimd.collective_compute(
            "AllGather", OP.bypass, replica_groups=GROUPS,
            ins=[kv_bnc[half].opt()], outs=[kv_all[half].opt()])
    unscope(sc)

    # q projection (no collective dependency)
    sc = scope("qproj_unpack")
    for half in range(2):
        kq_proj_half(wq_v, half, qT_sb, bq_fm)

    # unpack gathered K^T / V(+ones) into SBUF, per half
    for half in range(2):
        ga = kv_all[half][:]
        for c in range(CORES_PER_B):
            ksrc = bass.AP(tensor=ga.tensor,
                           offset=ga.offset + c * 1024 * TOK,
                           ap=[[TOK, 128], [128 * TOK, 4], [1, TOK]])
            nc.sync.dma_start(out=KT[:, half * 4:(half + 1) * 4, c, :],
                              in_=ksrc)
        vaa = Vaug[:]
        for hh in range(H // 2):
            h = half * 8 + hh
            for c in range(CORES_PER_B):
                vsrc = bass.AP(
                    tensor=ga.tensor,
                    offset=ga.offset + (c * 1024 + 512) * TOK + hh * 64,
                    ap=[[512, 128], [128 * 512, 4], [1, 64]])
                vdst2 = bass.AP(
                    tensor=vaa.tensor,
                    offset=vaa.offset + h * 65 + c * 4 * H * 65,
                    ap=[list(vaa.ap[0]), [H * 65, 4], [1, 64]])
                nc.sync.dma_start(out=vdst2, in_=vsrc)
            nc.vector.memset(Vaug[:, :, h * 65 + 64:h * 65 + 65], 1.0)
    es_3.close()
    es_X.close()
    unscope(sc)

    # prefetch wo (pC, used in out_proj) and w1 (pD, used in ffn1)
    nc.sync.dma_start(out=wo_sb[:], in_=wo_in.rearrange("(c p) f -> p c f", p=128))
    nc.sync.dma_start(out=w1_sb[:], in_=w1_in.rearrange("(c p) f -> p c f", p=128))

    # ---- stage 3: attention ---------------------------------------------
    sc = scope("attn")
    # x residual for stage 4 arrives into x2 during attention
    for i in range(TCH):
        nc.sync.dma_start(out=x2[:, i, :], in_=x_v[:, i, :])
    es_5 = ExitStack()
    ps_s = es_5.enter_context(tc.tile_pool(name="ps_s", bufs=3, space="PSUM"))
    ps_av = es_5.enter_context(tc.tile_pool(name="ps_av", bufs=2, space="PSUM"))
    s5e = es_5.enter_context(tc.tile_pool(name="s5e", bufs=14))
    s5i = es_5.enter_context(tc.tile_pool(name="s5i", bufs=10))

    exp_aps = {}

    def emit_scores_exp(hp):
        for g in range(NKC // 2):
            pss = [ps_s.tile([128, 2, 512], F32, tag="pss",
                             name=f"pss{hp}_{g}_{i}") for i in range(2)]
            for j in range(2):
                kc = 2 * g + j
                c, tcc = divmod(kc, 4)
                ksl = KT[:, hp, c, tcc * 128:(tcc + 1) * 128]
                nc.tensor.matmul(pss[0][:, j, :], lhsT=ksl[0:64, :],
                                 rhs=qT_sb[0:64, hp, :], start=True, stop=True,
                                 tile_position=(0, 0))
                nc.tensor.matmul(pss[1][:, j, :], lhsT=ksl[64:128, :],
                                 rhs=qT_sb[64:128, hp, :], start=True, stop=True,
                                 tile_position=(64, 0))
            for jh in range(2):
                # exp split: 10 tiles on Scalar (table exp), 6 on Vector
                if jh == 0 or g < 2:
                    e = s5e.tile([128, 2, 512], F8, tag="exp",
                                 name=f"e{hp}_{g}_{jh}")
                    nc.scalar.activation(out=e[:], in_=pss[jh][:], func=AF.Exp,
                                         scale=INV_SQRT_HD)
                    exp_aps[(hp, jh, g)] = e[:]
                else:
                    ei = s5i.tile([128, 2, 512], I8, tag="expi",
                                  name=f"ei{hp}_{g}_{jh}")
                    nc.vector.tensor_scalar(
                        out=ei[:], in0=pss[jh][:],
                        scalar1=FEXP_A * INV_SQRT_HD, scalar2=FEXP_B,
                        op0=OP.mult, op1=OP.add)
                    exp_aps[(hp, jh, g)] = ei[:].bitcast(F8)

    def emit_av(hp):
        for jh in range(2):
            h = 2 * hp + jh
            pav = ps_av.tile([128, 512], F32, tag="pav")
            for g in range(NKC // 2):
                nc.tensor.matmul(pav[0:65, :],
                                 lhsT=Vaug[:, 2 * g:2 * g + 2, h * 65:h * 65 + 65],
                                 rhs=exp_aps[(hp, jh, g)],
                                 start=(g == 0), stop=(g == NKC // 2 - 1),
                                 perf_mode=DR)
            nc.vector.tensor_copy(out=oun_sb[jh * 64:jh * 64 + 64, hp, :],
                                  in_=pav[0:64, :])
            nc.vector.tensor_copy(out=den_sb[0:1, h, :], in_=pav[64:65, :])
            for g in range(NKC // 2):
                del exp_aps[(hp, jh, g)]

    def emit_den_quarter(qr):
        """heads [4*qr, 4*qr+4) -> den_dram -> broadcast -> recip -> oT."""
        nc.sync.dma_start(out=den_dram[qr * 4:(qr + 1) * 4, :],
                          in_=den_sb[0:1, qr * 4:(qr + 1) * 4, :])
        dd_ap = den_dram[:]
        for pq in range(2):   # even heads -> partitions 0:64, odd -> 64:128
            bsrc = bass.AP(
                tensor=dd_ap.tensor,
                offset=dd_ap.offset + (qr * 4 + pq) * TOK,
                ap=[[0, 64], [2 * TOK, H // 8], [1, TOK]])
            nc.sync.dma_start(out=den_scr[pq * 64:(pq + 1) * 64, :, :], in_=bsrc)
        nc.vector.tensor_copy(out=den_bcf[:], in_=den_scr[:])
        nc.vector.reciprocal_approx_fast(out=den_inv[:], in_=den_bcf[:])
        nc.vector.tensor_copy(out=den_scr[:], in_=den_inv[:])
        nc.vector.tensor_tensor(
            out=oT[:, qr * 2:(qr + 1) * 2, :],
            in0=oun_sb[:, qr * 2:(qr + 1) * 2, :],
            in1=den_scr[:], op=OP.mult)

    # software pipeline: scores/exp one head-pair ahead of AV
    emit_scores_exp(0)
    for hp in range(1, H // 2):
        emit_scores_exp(hp)
        emit_av(hp - 1)
        if hp % 2 == 0:
            emit_den_quarter(hp // 2 - 1)
    emit_av(H // 2 - 1)
    emit_den_quarter(2)
    emit_den_quarter(3)
    es_5.close()
    es_B.close()
    unscope(sc)

    # ---- stage 4: out_proj + residual -----------------------------------
    # First 3 of 4 DoubleRow steps (head pairs 0-5, den quarters 0-2) are
    # emitted first so they overlap the attention tail; the last step joins
    # after den quarter 3.  LN2 stats interleave with the epilogue.
    sc = scope("outproj")
    es_6 = ExitStack()
    ps6 = es_6.enter_context(tc.tile_pool(name="ps6", bufs=8, space="PSUM"))
    ops = {}
    for to in range(TCH):
        for f2 in range(2):
            ps = ops[(to, f2)] = ps6.tile([128, 512], F32, tag="ps6",
                                          name=f"ops{to}_{f2}")
            for d2 in range(3):
                nc.tensor.matmul(ps[:],
                                 lhsT=oT[:, 2 * d2:2 * d2 + 2, to * 128:(to + 1) * 128],
                                 rhs=wo_sb[:, 2 * d2:2 * d2 + 2, f2 * 512:(f2 + 1) * 512],
                                 start=(d2 == 0), stop=False,
                                 perf_mode=DR)
    mvs2 = stats.tile([128, TCH, 2], F32, tag="mvs_ln2", name="mvs_ln2")
    for to in range(TCH):
        for f2 in range(2):
            ps = ops[(to, f2)]
            nc.tensor.matmul(ps[:],
                             lhsT=oT[:, 6:8, to * 128:(to + 1) * 128],
                             rhs=wo_sb[:, 6:8, f2 * 512:(f2 + 1) * 512],
                             start=False, stop=True, perf_mode=DR)
            dst = x2[:, to, f2 * 512:(f2 + 1) * 512]
            nc.vector.tensor_add(out=dst, in0=ps[:], in1=dst)
            if biases["out_proj_b"]:
                nc.vector.tensor_add(out=dst, in0=dst,
                                     in1=bo_bc[:, f2 * 512:(f2 + 1) * 512])
        st = stats.tile([128, 2, 6], F32, tag="bnst")
        nc.vector.bn_stats(out=st[:, 0, :], in_=x2[:, to, 0:512])
        nc.vector.bn_stats(out=st[:, 1, :], in_=x2[:, to, 512:1024])
        nc.vector.bn_aggr(out=mvs2[:, to, :], in_=st[:])
    es_6.close()
    es_C.close()
    unscope(sc)

    # ---- stage 5: LN2 + PE-transpose to hqT (fp8) ------------------------
    sc = scope("ln2q")
    es_G = ExitStack()
    pG = es_G.enter_context(tc.tile_pool(name="pG", bufs=1))     # y1gT, w2
    y1gT = pG.tile([128, FFCH, TOK], F8, tag="y1gT")
    w2_sb = pG.tile([128, FFCH, D], F8, tag="w2")
    nc.sync.dma_start(out=w2_sb[:], in_=w2_in.rearrange("(c p) f -> p c f", p=128))
    es_7 = ExitStack()
    s7 = es_7.enter_context(tc.tile_pool(name="s7", bufs=3))
    ps_t2 = es_7.enter_context(tc.tile_pool(name="ps_t2", bufs=4, space="PSUM"))
    rstd2 = stats.tile([128, TCH], F32, tag="rstd_ln2", name="rstd_ln2")
    var2_v = bass.AP(tensor=mvs2[:].tensor, offset=mvs2[:].offset + 1,
                     ap=[list(mvs2[:].ap[0])] + [[2, TCH]])
    nc.scalar.activation(out=rstd2[:], in_=var2_v, func=AF.Ln, bias=eps_t[:])
    nc.scalar.activation(out=rstd2[:], in_=rstd2[:], func=AF.Exp, scale=-0.5)
    for to in range(TCH):
        ht = s7.tile([128, D], BF16, tag="h")
        ln_norm_chunk(x2[:, to, :], mvs2, rstd2, to, g2_bc, b2ln_bc, ht[:])
        for dc in range(DCH):
            pt = ps_t2.tile([128, 128], BF16, tag="pt2")
            nc.tensor.transpose(pt[:], ht[:, dc * 128:(dc + 1) * 128], ident[:])
            nc.vector.tensor_copy(out=hqT[:, dc, to * 128:(to + 1) * 128], in_=pt[:])
    es_7.close()
    unscope(sc)

    # ---- stage 6: FFN mm1 (feature-major, fp8 DoubleRow) + gelu ----------
    sc = scope("ffn1")
    es_8 = ExitStack()
    ps8 = es_8.enter_context(tc.tile_pool(name="ps8", bufs=4, space="PSUM"))
    for fc in range(FFCH):
        ps = ps8.tile([128, 512], F32, tag="ps8")
        for d2 in range(DCH // 2):
            nc.tensor.matmul(ps[:],
                             lhsT=w1_sb[:, 2 * d2:2 * d2 + 2, fc * 128:(fc + 1) * 128],
                             rhs=hqT[:, 2 * d2:2 * d2 + 2, :],
                             start=(d2 == 0), stop=(d2 == DCH // 2 - 1),
                             perf_mode=DR)
        bias_ap = bf1_fm[:, fc:fc + 1] if biases["b1"] else 0.0
        nc.scalar.activation(out=y1gT[:, fc, :], in_=ps[:], func=gelu_func,
                             scale=float(s1), bias=bias_ap)
    es_8.close()
    unscope(sc)

    # ---- stage 7: FFN mm2 (fp8 DoubleRow) + residual -> out --------------
    sc = scope("ffn2")
    es_9 = ExitStack()
    ps9 = es_9.enter_context(tc.tile_pool(name="ps9", bufs=4, space="PSUM"))
    s9 = es_9.enter_context(tc.tile_pool(name="s9", bufs=3))
    out_v = out_d.rearrange("(i p) d -> p i d", p=128)
    for f2 in range(2):
        for to in range(TCH):
            ps = ps9.tile([128, 512], F32, tag="ps9")
            for c2 in range(FFCH // 2):
                nc.tensor.matmul(ps[:],
                                 lhsT=y1gT[:, 2 * c2:2 * c2 + 2, to * 128:(to + 1) * 128],
                                 rhs=w2_sb[:, 2 * c2:2 * c2 + 2, f2 * 512:(f2 + 1) * 512],
                                 start=(c2 == 0), stop=(c2 == FFCH // 2 - 1),
                                 perf_mode=DR)
            outt = s9.tile([128, 512], F32, tag="outt")
            nc.vector.scalar_tensor_tensor(
                out=outt[:], in0=ps[:], scalar=float(s2),
                in1=x2[:, to, f2 * 512:(f2 + 1) * 512], op0=OP.mult, op1=OP.add)
            if biases["b2"]:
                nc.vector.tensor_add(out=outt[:], in0=outt[:],
                                     in1=bf2_bc[:, f2 * 512:(f2 + 1) * 512])
            nc.sync.dma_start(out=out_v[:, to, f2 * 512:(f2 + 1) * 512],
                              in_=outt[:])
    es_9.close()
    unscope(sc)
    es_G.close()
    es_E.close()
    es_D.close()
    es_top.close()


_CACHE = {}


def _prepare(inputs):
    bf = ml_dtypes.bfloat16
    f8 = ml_dtypes.float8_e4m3
    x = np.ascontiguousarray(np.asarray(inputs["x"], dtype=np.float32))
    in_w = np.asarray(inputs["in_proj_w"], dtype=np.float32)
    out_w = np.asarray(inputs["out_proj_w"], dtype=np.float32)
    w1 = np.asarray(inputs["w1"], dtype=np.float32)
    w2 = np.asarray(inputs["w2"], dtype=np.float32)

    s1 = float(max(np.mean(np.abs(w1), dtype=np.float32), EPS))
    s2 = float(max(np.mean(np.abs(w2), dtype=np.float32), EPS))
    t1 = np.clip(np.round(w1 / np.float32(s1)), -1.0, 1.0).astype(np.float32)
    t2 = np.clip(np.round(w2 / np.float32(s2)), -1.0, 1.0).astype(np.float32)

    host = {
        "wqT": np.ascontiguousarray(in_w[0:D].T).astype(f8),
        "wkT": np.ascontiguousarray(in_w[D:2 * D].T).astype(f8),
        "wvT": np.ascontiguousarray(in_w[2 * D:3 * D].T).astype(f8),
        "woT": np.ascontiguousarray(out_w.T).astype(f8),
        "w1T": np.ascontiguousarray(t1.T).astype(f8),
        "w2T": np.ascontiguousarray(t2.T).astype(f8),
        "ident": np.eye(128, dtype=np.float32).astype(bf),
    }

    def nz(a):
        return bool(np.any(np.asarray(a) != 0.0))

    biases = {
        "ln1_g": bool(np.any(np.asarray(inputs["ln1_g"]) != 1.0)),
        "ln1_b": nz(inputs["ln1_b"]),
        "ln2_g": bool(np.any(np.asarray(inputs["ln2_g"]) != 1.0)),
        "ln2_b": nz(inputs["ln2_b"]),
        "in_proj_b": nz(inputs["in_proj_b"]),
        "out_proj_b": nz(inputs["out_proj_b"]),
        "b1": nz(inputs["b1"]),
        "b2": nz(inputs["b2"]),
    }
    extra = {}
    if biases["ln1_g"]:
        extra["ln1_g"] = np.asarray(inputs["ln1_g"], np.float32)
    if biases["ln1_b"]:
        extra["ln1_b"] = np.asarray(inputs["ln1_b"], np.float32)
    if biases["ln2_g"]:
        extra["ln2_g"] = np.asarray(inputs["ln2_g"], np.float32)
    if biases["ln2_b"]:
        extra["ln2_b"] = np.asarray(inputs["ln2_b"], np.float32)
    if biases["in_proj_b"]:
        extra["in_b"] = np.asarray(inputs["in_proj_b"], np.float32)
    if biases["out_proj_b"]:
        extra["out_b"] = np.asarray(inputs["out_proj_b"], np.float32)
    if biases["b1"]:
        extra["b1"] = np.asarray(inputs["b1"], np.float32)
    if biases["b2"]:
        extra["b2"] = np.asarray(inputs["b2"], np.float32)

    x_flat = x.reshape(NTOK, D)
    in_maps = []
    for c in range(N_CORES):
        m = {"x_sh": np.ascontiguousarray(x_flat[c * TOK:(c + 1) * TOK])}
        m.update(host)
        m.update(extra)
        in_maps.append(m)
    return in_maps, s1, s2, biases


def get_program(s1, s2, biases, for_hw=True, sim_gelu=False):
    key = (round(s1, 12), round(s2, 12), tuple(sorted(biases.items())), for_hw,
           sim_gelu)
    if key not in _CACHE:
        nc = build_program(s1, s2, biases, sim_gelu=sim_gelu)
        if for_hw:
            nc.m = get_hw_module(nc.m)
        _CACHE[key] = nc
    return _CACHE[key]


def kernel(**inputs):
    in_maps, s1, s2, biases = _prepare(inputs)
    nc = get_program(s1, s2, biases, for_hw=True)
    res = run_bass_kernel_spmd(nc, in_maps, list(range(N_CORES)))
    out = np.concatenate([res.results[c]["out"] for c in range(N_CORES)], axis=0)
    return out.reshape(B, S, D).astype(np.float32)


# revision 16
# speedup vs baseline: 1.1246x; 1.1246x over previous
"""BitTransformerBlock on 8 Trainium2 NeuronCores.

Token-parallel sharding: the flattened (B*S)=4096 tokens are split 512 per
core; cores 0-3 hold batch 0, cores 4-7 batch 1.  Each core computes LN1 and
the q/k/v projections for its own tokens; four small in-kernel AllGathers
(k-lo, v-lo, k-hi, v-hi; replica groups [0..3], [4..7]) share K and V across
each batch group pipelined against the projections, and everything
downstream (attention over the full 2048-token context, out-proj, LN2, the
FFN) is token-local.

Numerics: most matmuls run fp8(e4m3) with DoubleRow (2 contraction rows per
PE cell, half the matmul instructions); scores stay bf16.  The BitNet
act_quant round-trips are skipped entirely (quantization noise is ~1e-3 on
the final output); the host-ternarized weights {-1,0,1} are fp8-exact and
their scales s1/s2 fold into the gelu scale and the final residual fma.
Softmax runs without max subtraction (logits are small); exp is split
between the Scalar engine (table exp -> fp8) and the Vector engine
(Schraudolph fast-exp: fma to int8, bitcast to fp8e4m3).  All transposes
are PE transposes through PSUM.  Softmax denominators are batched per
half: DRAM bounce, partition-broadcast DMA, fast reciprocal, one multiply.
"""

import numpy as np
import ml_dtypes

import concourse.bacc as bacc
import concourse.bass as bass
import concourse.mybir as mybir
import concourse.tile as tile
from concourse.bass_interp import get_hw_module
from concourse.bass_utils import run_bass_kernel_spmd

F32 = mybir.dt.float32
BF16 = mybir.dt.bfloat16
F8 = mybir.dt.float8e4
I16 = mybir.dt.int16
I8 = mybir.dt.int8
AF = mybir.ActivationFunctionType
OP = mybir.AluOpType
DR = mybir.MatmulPerfMode.DoubleRow

N_CORES = 8
B, S, D, H, FF = 2, 2048, 1024, 16, 4096
HD = D // H                 # 64
NTOK = B * S                # 4096
TOK = NTOK // N_CORES       # 512 tokens per core
TCH = TOK // 128            # 4 token chunks per core
DCH = D // 128              # 8
FFCH = FF // 128            # 32
NKC = S // 128              # 16 key chunks per batch
GROUPS = [[0, 1, 2, 3], [4, 5, 6, 7]]
CORES_PER_B = 4
EPS = 1e-5
INV_SQRT_HD = 1.0 / 8.0
# Schraudolph fast-exp in fp8e4m3: bits8 = round(x * 2^3/ln2 + (7-c)*2^3)
FEXP_A = float(2.0 ** 3 / np.log(2.0))
FEXP_B = float((7.0 - 0.043) * 2 ** 3)


def _bcast_part(ap, parts):
    """View a [1, F] (or [F]) AP as [parts, F] via a zero-stride partition dim."""
    inner = [list(e) for e in ap.ap if e[1] != 1] or [[1, 1]]
    return bass.AP(tensor=ap.tensor, offset=ap.offset, ap=[[0, parts]] + inner)


def build_program(s1, s2, biases, sim_gelu=False):
    nc = bacc.Bacc("TRN2", target_bir_lowering=False, debug=False,
                   num_devices=N_CORES)

    x_in = nc.dram_tensor("x_sh", [TOK, D], F32, kind="ExternalInput")
    wq_in = nc.dram_tensor("wqT", [D, D], F8, kind="ExternalInput")
    wk_in = nc.dram_tensor("wkT", [D, D], F8, kind="ExternalInput")
    wv_in = nc.dram_tensor("wvT", [D, D], F8, kind="ExternalInput")
    wo_in = nc.dram_tensor("woT", [D, D], F8, kind="ExternalInput")
    w1_in = nc.dram_tensor("w1T", [D, FF], F8, kind="ExternalInput")
    w2_in = nc.dram_tensor("w2T", [FF, D], F8, kind="ExternalInput")
    id_in = nc.dram_tensor("ident", [128, 128], BF16, kind="ExternalInput")
    out_d = nc.dram_tensor("out", [TOK, D], F32, kind="ExternalOutput")

    ext = {}
    if biases["ln1_g"]:
        ext["ln1_g"] = nc.dram_tensor("ln1_g", [D], F32, kind="ExternalInput")
    if biases["ln1_b"]:
        ext["ln1_b"] = nc.dram_tensor("ln1_b", [D], F32, kind="ExternalInput")
    if biases["ln2_g"]:
        ext["ln2_g"] = nc.dram_tensor("ln2_g", [D], F32, kind="ExternalInput")
    if biases["ln2_b"]:
        ext["ln2_b"] = nc.dram_tensor("ln2_b", [D], F32, kind="ExternalInput")
    if biases["in_proj_b"]:
        ext["in_b"] = nc.dram_tensor("in_b", [3 * D], F32, kind="ExternalInput")
    if biases["out_proj_b"]:
        ext["out_b"] = nc.dram_tensor("out_b", [D], F32, kind="ExternalInput")
    if biases["b1"]:
        ext["b1"] = nc.dram_tensor("b1", [FF], F32, kind="ExternalInput")
    if biases["b2"]:
        ext["b2"] = nc.dram_tensor("b2", [D], F32, kind="ExternalInput")

    with tile.TileContext(nc) as tc:
        _emit(nc, tc, x_in, wq_in, wk_in, wv_in, wo_in, w1_in, w2_in, id_in,
              out_d, ext, s1, s2, biases, sim_gelu)
    nc.compile()
    return nc


def _emit(nc, tc, x_in, wq_in, wk_in, wv_in, wo_in, w1_in, w2_in, id_in,
          out_d, ext, s1, s2, biases, sim_gelu=False):
    gelu_func = AF.Tanh if sim_gelu else AF.Gelu
    from contextlib import ExitStack

    def scope(name):
        sid = nc.enter_named_scope(name, False)
        return (name, sid[0] if isinstance(sid, tuple) else sid)

    def unscope(tok):
        nc.leave_named_scope(tok[0], tok[1], False)

    es_top = ExitStack()
    dram = es_top.enter_context(tc.tile_pool(name="dram", bufs=1, space="DRAM"))
    const = es_top.enter_context(tc.tile_pool(name="const", bufs=1))
    stats = es_top.enter_context(tc.tile_pool(name="stats", bufs=4))

    kT_bnc = [dram.tile([D // 2, TOK], F8, name=f"kT_bnc{i}")
              for i in range(2)]
    v_bnc = [dram.tile([TOK, D // 2], F8, name=f"v_bnc{i}") for i in range(2)]
    kT_all = [dram.tile([CORES_PER_B * (D // 2), TOK], F8, name=f"kT_all{i}")
              for i in range(2)]
    v_all = [dram.tile([S, D // 2], F8, name=f"v_all{i}") for i in range(2)]
    den_dram = dram.tile([H, TOK], BF16)

    eps_t = const.tile([128, 1], F32)
    nc.vector.memset(eps_t[:], EPS)
    ident = const.tile([128, 128], BF16, tag="ident")
    nc.sync.dma_start(out=ident[:], in_=id_in[:])

    def load_bcast(name, width, src_ap):
        t = const.tile([128, width], F32, tag=f"bc_{name}")
        nc.sync.dma_start(out=t[:], in_=_bcast_part(src_ap, 128))
        return t

    g1_bc = load_bcast("g1", D, ext["ln1_g"][:]) if biases["ln1_g"] else None
    b1ln_bc = load_bcast("b1ln", D, ext["ln1_b"][:]) if biases["ln1_b"] else None
    g2_bc = load_bcast("g2", D, ext["ln2_g"][:]) if biases["ln2_g"] else None
    b2ln_bc = load_bcast("b2ln", D, ext["ln2_b"][:]) if biases["ln2_b"] else None
    bv_bc = (load_bcast("bv", D, ext["in_b"][2 * D:3 * D])
             if biases["in_proj_b"] else None)
    bo_bc = load_bcast("bo", D, ext["out_b"][:]) if biases["out_proj_b"] else None
    bf2_bc = load_bcast("bf2", D, ext["b2"][:]) if biases["b2"] else None
    bq_fm = bk_fm = None
    if biases["in_proj_b"]:
        bq_fm = const.tile([128, DCH], F32, tag="bq_fm")
        nc.sync.dma_start(out=bq_fm[:], in_=ext["in_b"][0:D].rearrange("(c p) -> p c", p=128))
        bk_fm = const.tile([128, DCH], F32, tag="bk_fm")
        nc.sync.dma_start(out=bk_fm[:], in_=ext["in_b"][D:2 * D].rearrange("(c p) -> p c", p=128))
    if biases["b1"]:
        bf1_fm = const.tile([128, FFCH], F32, tag="bf1_fm")
        nc.sync.dma_start(out=bf1_fm[:], in_=ext["b1"][:].rearrange("(c p) -> p c", p=128))

    # ---- pool stack (lifetimes nest: later-opened closes first) ----------
    es_D = ExitStack()
    pD = es_D.enter_context(tc.tile_pool(name="pD", bufs=1))     # x2, w1
    es_E = ExitStack()
    pE = es_E.enter_context(tc.tile_pool(name="pE", bufs=1))     # hqT
    es_C = ExitStack()
    pC = es_C.enter_context(tc.tile_pool(name="pC", bufs=1))     # oun/den/wo/oT
    es_B = ExitStack()
    pB = es_B.enter_context(tc.tile_pool(name="pB", bufs=1))     # KT/Vaug/qT
    es_X = ExitStack()
    pX = es_X.enter_context(tc.tile_pool(name="pX", bufs=1))     # nxT

    x2 = pD.tile([128, TCH, D], F32, tag="x2")
    w1_sb = pD.tile([128, DCH, FF], F8, tag="w1")
    hqT = pE.tile([128, DCH, TOK], F8, tag="hqT")
    oun_sb = pC.tile([128, H // 2, TOK], BF16, tag="oun")
    den_sb = pC.tile([1, H, TOK], BF16, tag="den")
    wo_sb = pC.tile([128, DCH, D], F8, tag="wo")
    oT = pC.tile([128, DCH, TOK], F8, tag="oT")
    den_scr = pC.tile([128, H // 8, TOK], BF16, tag="denscr")   # per-quarter
    den_bcf = pC.tile([128, H // 8, TOK], F32, tag="denbcf")
    den_inv = pC.tile([128, H // 8, TOK], F32, tag="deninv")
    KT = pB.tile([128, DCH, CORES_PER_B, 512], F8, tag="KT")
    Vaug = pB.tile([128, NKC, H * (HD + 1)], F8, tag="Va")
    qT_sb = pB.tile([128, DCH, TOK], F8, tag="qT")
    nxT = pX.tile([128, DCH, TOK], F8, tag="nxT")

    def ln_stats(src_tile, tag):
        """All-chunk LN stats: one Ln + one Exp.  Returns (mvs, rstd)."""
        mvs = stats.tile([128, TCH, 2], F32, tag=f"mvs_{tag}", name=f"mvs_{tag}")
        for c in range(TCH):
            st = stats.tile([128, 2, 6], F32, tag="bnst")
            nc.vector.bn_stats(out=st[:, 0, :], in_=src_tile[:, c, 0:512])
            nc.vector.bn_stats(out=st[:, 1, :], in_=src_tile[:, c, 512:1024])
            nc.vector.bn_aggr(out=mvs[:, c, :], in_=st[:])
        rstd = stats.tile([128, TCH], F32, tag=f"rstd_{tag}", name=f"rstd_{tag}")
        var_v = bass.AP(tensor=mvs[:].tensor, offset=mvs[:].offset + 1,
                        ap=[list(mvs[:].ap[0])] + [[2, TCH]])
        nc.scalar.activation(out=rstd[:], in_=var_v, func=AF.Ln, bias=eps_t[:])
        nc.scalar.activation(out=rstd[:], in_=rstd[:], func=AF.Exp, scale=-0.5)
        return mvs, rstd

    def ln_norm_chunk(src_ap, mvs, rstd, c, g_bc, b_bc, out_tile):
        nc.vector.tensor_scalar(out=out_tile, in0=src_ap,
                                scalar1=mvs[:, c, 0:1], scalar2=rstd[:, c:c + 1],
                                op0=OP.subtract, op1=OP.mult)
        if g_bc is not None:
            nc.vector.tensor_mul(out=out_tile, in0=out_tile, in1=g_bc[:])
        if b_bc is not None:
            nc.vector.tensor_add(out=out_tile, in0=out_tile, in1=b_bc[:])

    # ---- stage 1: load x, LN1, PE-transpose to nxT (fp8) -----------------
    sc = scope("ln1")
    es_1 = ExitStack()
    s1p = es_1.enter_context(tc.tile_pool(name="s1p", bufs=1))
    s1n = es_1.enter_context(tc.tile_pool(name="s1n", bufs=3))
    ps_t = es_1.enter_context(tc.tile_pool(name="ps_t", bufs=4, space="PSUM"))
    x_v = x_in.rearrange("(i p) d -> p i d", p=128)
    x_sb = s1p.tile([128, TCH, D], F32, tag="xsb", name="x_sb")
    for i in range(TCH):
        nc.sync.dma_start(out=x_sb[:, i, :], in_=x_v[:, i, :])
    mvs1, rstd1 = ln_stats(x_sb, "ln1")
    for i in range(TCH):
        nxt = s1n.tile([128, D], BF16, tag="nx")
        ln_norm_chunk(x_sb[:, i, :], mvs1, rstd1, i, g1_bc, b1ln_bc, nxt[:])
        for dc in range(DCH):
            pt = ps_t.tile([128, 128], BF16, tag="pt")
            nc.tensor.transpose(pt[:], nxt[:, dc * 128:(dc + 1) * 128], ident[:])
            nc.vector.tensor_copy(out=nxT[:, dc, i * 128:(i + 1) * 128], in_=pt[:])
    es_1.close()
    unscope(sc)

    # ---- stage 2: k/v/q projections + 4 pipelined AllGathers -------------
    sc = scope("inproj")
    es_3 = ExitStack()
    pW = es_3.enter_context(tc.tile_pool(name="pW", bufs=3))
    ps3 = es_3.enter_context(tc.tile_pool(name="ps3", bufs=4, space="PSUM"))
    s3 = es_3.enter_context(tc.tile_pool(name="s3", bufs=1))

    wk_v = wk_in.rearrange("(c p) f -> p c f", p=128)
    wq_v = wq_in.rearrange("(c p) f -> p c f", p=128)
    wv_v = wv_in.rearrange("(c p) f -> p c f", p=128)

    kT_loc = s3.tile([128, DCH, 512], F8, tag="kT_loc", name="kT_loc")
    v_loc = s3.tile([128, TCH, D], F8, tag="v_loc", name="v_loc")

    def kq_proj_half(w_view, half, out_tile, bias_fm):
        wt = pW.tile([128, DCH, 512], F8, tag="wslc")
        nc.sync.dma_start(out=wt[:], in_=w_view[:, :, half * 512:(half + 1) * 512])
        for fi in range(4):
            fo = half * 4 + fi
            ps = ps3.tile([128, 512], F32, tag="ps")
            for d2 in range(DCH // 2):
                nc.tensor.matmul(ps[:],
                                 lhsT=wt[:, 2 * d2:2 * d2 + 2, fi * 128:(fi + 1) * 128],
                                 rhs=nxT[:, 2 * d2:2 * d2 + 2, :],
                                 start=(d2 == 0), stop=(d2 == DCH // 2 - 1),
                                 perf_mode=DR)
            if biases["in_proj_b"]:
                nc.scalar.activation(out=out_tile[:, fo, :], in_=ps[:],
                                     func=AF.Identity, bias=bias_fm[:, fo:fo + 1])
            else:
                nc.scalar.activation(out=out_tile[:, fo, :], in_=ps[:], func=AF.Copy)

    def v_proj_half(f2):
        wt = pW.tile([128, DCH, 512], F8, tag="wslc")
        nc.sync.dma_start(out=wt[:], in_=wv_v[:, :, f2 * 512:(f2 + 1) * 512])
        for to in range(TCH):
            ps = ps3.tile([128, 512], F32, tag="ps")
            for d2 in range(DCH // 2):
                nc.tensor.matmul(ps[:],
                                 lhsT=nxT[:, 2 * d2:2 * d2 + 2, to * 128:(to + 1) * 128],
                                 rhs=wt[:, 2 * d2:2 * d2 + 2, :],
                                 start=(d2 == 0), stop=(d2 == DCH // 2 - 1),
                                 perf_mode=DR)
            dst = v_loc[:, to, f2 * 512:(f2 + 1) * 512]
            if biases["in_proj_b"]:
                nc.vector.tensor_add(out=dst, in0=ps[:],
                                     in1=bv_bc[:, f2 * 512:(f2 + 1) * 512])
            else:
                nc.vector.tensor_copy(out=dst, in_=ps[:])

    for half in range(2):
        kq_proj_half(wk_v, half, kT_loc, bk_fm)
        nc.sync.dma_start(
            out=kT_bnc[half].rearrange("(c p) t -> p c t", p=128),
            in_=kT_loc[:, half * 4:(half + 1) * 4, :])
        nc.gpsimd.collective_compute(
            "AllGather", OP.bypass, replica_groups=GROUPS,
            ins=[kT_bnc[half].opt()], outs=[kT_all[half].opt()])
        v_proj_half(half)
        nc.sync.dma_start(
            out=v_bnc[half].rearrange("(i p) f -> p i f", p=128),
            in_=v_loc[:, :, half * 512:(half + 1) * 512])
        nc.gpsimd.collective_compute(
            "AllGather", OP.bypass, replica_groups=GROUPS,
            ins=[v_bnc[half].opt()], outs=[v_all[half].opt()])
    unscope(sc)

    # q projection (no collective dependency)
    sc = scope("qproj_unpack")
    for half in range(2):
        kq_proj_half(wq_v, half, qT_sb, bq_fm)

    # unpack gathered K^T / V(+ones) into SBUF, per half
    for half in range(2):
        for c in range(CORES_PER_B):
            nc.sync.dma_start(
                out=KT[:, half * 4:(half + 1) * 4, c, :],
                in_=kT_all[half][c * 512:(c + 1) * 512, :]
                .rearrange("(dch p) t -> p dch t", p=128))
        va_v = v_all[half].rearrange("(kc p) f -> p kc f", p=128)
        for hh in range(H // 2):
            h = half * 8 + hh
            nc.sync.dma_start(out=Vaug[:, :, h * 65:h * 65 + 64],
                              in_=va_v[:, :, hh * 64:(hh + 1) * 64])
            nc.vector.memset(Vaug[:, :, h * 65 + 64:h * 65 + 65], 1.0)
    es_3.close()
    es_X.close()
    unscope(sc)

    # prefetch wo (pC, used in out_proj) and w1 (pD, used in ffn1)
    nc.sync.dma_start(out=wo_sb[:], in_=wo_in.rearrange("(c p) f -> p c f", p=128))
    nc.sync.dma_start(out=w1_sb[:], in_=w1_in.rearrange("(c p) f -> p c f", p=128))

    # ---- stage 3: attention ---------------------------------------------
    sc = scope("attn")
    # x residual for stage 4 arrives into x2 during attention
    for i in range(TCH):
        nc.sync.dma_start(out=x2[:, i, :], in_=x_v[:, i, :])
    es_5 = ExitStack()
    ps_s = es_5.enter_context(tc.tile_pool(name="ps_s", bufs=3, space="PSUM"))
    ps_av = es_5.enter_context(tc.tile_pool(name="ps_av", bufs=2, space="PSUM"))
    s5e = es_5.enter_context(tc.tile_pool(name="s5e", bufs=14))
    s5i = es_5.enter_context(tc.tile_pool(name="s5i", bufs=10))

    exp_aps = {}

    def emit_scores_exp(hp):
        for g in range(NKC // 2):
            pss = [ps_s.tile([128, 2, 512], F32, tag="pss",
                             name=f"pss{hp}_{g}_{i}") for i in range(2)]
            for j in range(2):
                kc = 2 * g + j
                c, tcc = divmod(kc, 4)
                ksl = KT[:, hp, c, tcc * 128:(tcc + 1) * 128]
                nc.tensor.matmul(pss[0][:, j, :], lhsT=ksl[0:64, :],
                                 rhs=qT_sb[0:64, hp, :], start=True, stop=True,
                                 tile_position=(0, 0))
                nc.tensor.matmul(pss[1][:, j, :], lhsT=ksl[64:128, :],
                                 rhs=qT_sb[64:128, hp, :], start=True, stop=True,
                                 tile_position=(64, 0))
            for jh in range(2):
                # exp split: 10 tiles on Scalar (table exp), 6 on Vector
                if jh == 0 or g < 2:
                    e = s5e.tile([128, 2, 512], F8, tag="exp",
                                 name=f"e{hp}_{g}_{jh}")
                    nc.scalar.activation(out=e[:], in_=pss[jh][:], func=AF.Exp,
                                         scale=INV_SQRT_HD)
                    exp_aps[(hp, jh, g)] = e[:]
                else:
                    ei = s5i.tile([128, 2, 512], I8, tag="expi",
                                  name=f"ei{hp}_{g}_{jh}")
                    nc.vector.tensor_scalar(
                        out=ei[:], in0=pss[jh][:],
                        scalar1=FEXP_A * INV_SQRT_HD, scalar2=FEXP_B,
                        op0=OP.mult, op1=OP.add)
                    exp_aps[(hp, jh, g)] = ei[:].bitcast(F8)

    def emit_av(hp):
        for jh in range(2):
            h = 2 * hp + jh
            pav = ps_av.tile([128, 512], F32, tag="pav")
            for g in range(NKC // 2):
                nc.tensor.matmul(pav[0:65, :],
                                 lhsT=Vaug[:, 2 * g:2 * g + 2, h * 65:h * 65 + 65],
                                 rhs=exp_aps[(hp, jh, g)],
                                 start=(g == 0), stop=(g == NKC // 2 - 1),
                                 perf_mode=DR)
            nc.vector.tensor_copy(out=oun_sb[jh * 64:jh * 64 + 64, hp, :],
                                  in_=pav[0:64, :])
            nc.vector.tensor_copy(out=den_sb[0:1, h, :], in_=pav[64:65, :])
            for g in range(NKC // 2):
                del exp_aps[(hp, jh, g)]

    def emit_den_quarter(qr):
        """heads [4*qr, 4*qr+4) -> den_dram -> broadcast -> recip -> oT."""
        nc.sync.dma_start(out=den_dram[qr * 4:(qr + 1) * 4, :],
                          in_=den_sb[0:1, qr * 4:(qr + 1) * 4, :])
        dd_ap = den_dram[:]
        for pq in range(2):   # even heads -> partitions 0:64, odd -> 64:128
            bsrc = bass.AP(
                tensor=dd_ap.tensor,
                offset=dd_ap.offset + (qr * 4 + pq) * TOK,
                ap=[[0, 64], [2 * TOK, H // 8], [1, TOK]])
            nc.sync.dma_start(out=den_scr[pq * 64:(pq + 1) * 64, :, :], in_=bsrc)
        nc.vector.tensor_copy(out=den_bcf[:], in_=den_scr[:])
        nc.vector.reciprocal_approx_fast(out=den_inv[:], in_=den_bcf[:])
        nc.vector.tensor_copy(out=den_scr[:], in_=den_inv[:])
        nc.vector.tensor_tensor(
            out=oT[:, qr * 2:(qr + 1) * 2, :],
            in0=oun_sb[:, qr * 2:(qr + 1) * 2, :],
            in1=den_scr[:], op=OP.mult)

    # software pipeline: scores/exp one head-pair ahead of AV
    emit_scores_exp(0)
    for hp in range(1, H // 2):
        emit_scores_exp(hp)
        emit_av(hp - 1)
        if hp % 2 == 0:
            emit_den_quarter(hp // 2 - 1)
    emit_av(H // 2 - 1)
    emit_den_quarter(2)
    emit_den_quarter(3)
    es_5.close()
    es_B.close()
    unscope(sc)

    # ---- stage 4: out_proj + residual -----------------------------------
    # First 3 of 4 DoubleRow steps (head pairs 0-5, den quarters 0-2) are
    # emitted first so they overlap the attention tail; the last step joins
    # after den quarter 3.  LN2 stats interleave with the epilogue.
    sc = scope("outproj")
    es_6 = ExitStack()
    ps6 = es_6.enter_context(tc.tile_pool(name="ps6", bufs=8, space="PSUM"))
    ops = {}
    for to in range(TCH):
        for f2 in range(2):
            ps = ops[(to, f2)] = ps6.tile([128, 512], F32, tag="ps6",
                                          name=f"ops{to}_{f2}")
            for d2 in range(3):
                nc.tensor.matmul(ps[:],
                                 lhsT=oT[:, 2 * d2:2 * d2 + 2, to * 128:(to + 1) * 128],
                                 rhs=wo_sb[:, 2 * d2:2 * d2 + 2, f2 * 512:(f2 + 1) * 512],
                                 start=(d2 == 0), stop=False,
                                 perf_mode=DR)
    mvs2 = stats.tile([128, TCH, 2], F32, tag="mvs_ln2", name="mvs_ln2")
    for to in range(TCH):
        for f2 in range(2):
            ps = ops[(to, f2)]
            nc.tensor.matmul(ps[:],
                             lhsT=oT[:, 6:8, to * 128:(to + 1) * 128],
                             rhs=wo_sb[:, 6:8, f2 * 512:(f2 + 1) * 512],
                             start=False, stop=True, perf_mode=DR)
            dst = x2[:, to, f2 * 512:(f2 + 1) * 512]
            nc.vector.tensor_add(out=dst, in0=ps[:], in1=dst)
            if biases["out_proj_b"]:
                nc.vector.tensor_add(out=dst, in0=dst,
                                     in1=bo_bc[:, f2 * 512:(f2 + 1) * 512])
        st = stats.tile([128, 2, 6], F32, tag="bnst")
        nc.vector.bn_stats(out=st[:, 0, :], in_=x2[:, to, 0:512])
        nc.vector.bn_stats(out=st[:, 1, :], in_=x2[:, to, 512:1024])
        nc.vector.bn_aggr(out=mvs2[:, to, :], in_=st[:])
    es_6.close()
    es_C.close()
    unscope(sc)

    # ---- stage 5: LN2 + PE-transpose to hqT (fp8) ------------------------
    sc = scope("ln2q")
    es_G = ExitStack()
    pG = es_G.enter_context(tc.tile_pool(name="pG", bufs=1))     # y1gT, w2
    y1gT = pG.tile([128, FFCH, TOK], F8, tag="y1gT")
    w2_sb = pG.tile([128, FFCH, D], F8, tag="w2")
    nc.sync.dma_start(out=w2_sb[:], in_=w2_in.rearrange("(c p) f -> p c f", p=128))
    es_7 = ExitStack()
    s7 = es_7.enter_context(tc.tile_pool(name="s7", bufs=3))
    ps_t2 = es_7.enter_context(tc.tile_pool(name="ps_t2", bufs=4, space="PSUM"))
    rstd2 = stats.tile([128, TCH], F32, tag="rstd_ln2", name="rstd_ln2")
    var2_v = bass.AP(tensor=mvs2[:].tensor, offset=mvs2[:].offset + 1,
                     ap=[list(mvs2[:].ap[0])] + [[2, TCH]])
    nc.scalar.activation(out=rstd2[:], in_=var2_v, func=AF.Ln, bias=eps_t[:])
    nc.scalar.activation(out=rstd2[:], in_=rstd2[:], func=AF.Exp, scale=-0.5)
    for to in range(TCH):
        ht = s7.tile([128, D], BF16, tag="h")
        ln_norm_chunk(x2[:, to, :], mvs2, rstd2, to, g2_bc, b2ln_bc, ht[:])
        for dc in range(DCH):
            pt = ps_t2.tile([128, 128], BF16, tag="pt2")
            nc.tensor.transpose(pt[:], ht[:, dc * 128:(dc + 1) * 128], ident[:])
            nc.vector.tensor_copy(out=hqT[:, dc, to * 128:(to + 1) * 128], in_=pt[:])
    es_7.close()
    unscope(sc)

    # ---- stage 6: FFN mm1 (feature-major, fp8 DoubleRow) + gelu ----------
    sc = scope("ffn1")
    es_8 = ExitStack()
    ps8 = es_8.enter_context(tc.tile_pool(name="ps8", bufs=4, space="PSUM"))
    for fc in range(FFCH):
        ps = ps8.tile([128, 512], F32, tag="ps8")
        for d2 in range(DCH // 2):
            nc.tensor.matmul(ps[:],
                             lhsT=w1_sb[:, 2 * d2:2 * d2 + 2, fc * 128:(fc + 1) * 128],
                             rhs=hqT[:, 2 * d2:2 * d2 + 2, :],
                             start=(d2 == 0), stop=(d2 == DCH // 2 - 1),
                             perf_mode=DR)
        bias_ap = bf1_fm[:, fc:fc + 1] if biases["b1"] else 0.0
        nc.scalar.activation(out=y1gT[:, fc, :], in_=ps[:], func=gelu_func,
                             scale=float(s1), bias=bias_ap)
    es_8.close()
    unscope(sc)

    # ---- stage 7: FFN mm2 (fp8 DoubleRow) + residual -> out --------------
    sc = scope("ffn2")
    es_9 = ExitStack()
    ps9 = es_9.enter_context(tc.tile_pool(name="ps9", bufs=4, space="PSUM"))
    s9 = es_9.enter_context(tc.tile_pool(name="s9", bufs=3))
    out_v = out_d.rearrange("(i p) d -> p i d", p=128)
    for f2 in range(2):
        for to in range(TCH):
            ps = ps9.tile([128, 512], F32, tag="ps9")
            for c2 in range(FFCH // 2):
                nc.tensor.matmul(ps[:],
                                 lhsT=y1gT[:, 2 * c2:2 * c2 + 2, to * 128:(to + 1) * 128],
                                 rhs=w2_sb[:, 2 * c2:2 * c2 + 2, f2 * 512:(f2 + 1) * 512],
                                 start=(c2 == 0), stop=(c2 == FFCH // 2 - 1),
                                 perf_mode=DR)
            outt = s9.tile([128, 512], F32, tag="outt")
            nc.vector.scalar_tensor_tensor(
                out=outt[:], in0=ps[:], scalar=float(s2),
                in1=x2[:, to, f2 * 512:(f2 + 1) * 512], op0=OP.mult, op1=OP.add)
            if biases["b2"]:
                nc.vector.tensor_add(out=outt[:], in0=outt[:],
                                     in1=bf2_bc[:, f2 * 512:(f2 + 1) * 512])
            nc.sync.dma_start(out=out_v[:, to, f2 * 512:(f2 + 1) * 512],
                              in_=outt[:])
    es_9.close()
    unscope(sc)
    es_G.close()
    es_E.close()
    es_D.close()
    es_top.close()


_CACHE = {}


def _prepare(inputs):
    bf = ml_dtypes.bfloat16
    f8 = ml_dtypes.float8_e4m3
    x = np.ascontiguousarray(np.asarray(inputs["x"], dtype=np.float32))
    in_w = np.asarray(inputs["in_proj_w"], dtype=np.float32)
    out_w = np.asarray(inputs["out_proj_w"], dtype=np.float32)
    w1 = np.asarray(inputs["w1"], dtype=np.float32)
    w2 = np.asarray(inputs["w2"], dtype=np.float32)

    s1 = float(max(np.mean(np.abs(w1), dtype=np.float32), EPS))
    s2 = float(max(np.mean(np.abs(w2), dtype=np.float32), EPS))
    t1 = np.clip(np.round(w1 / np.float32(s1)), -1.0, 1.0).astype(np.float32)
    t2 = np.clip(np.round(w2 / np.float32(s2)), -1.0, 1.0).astype(np.float32)

    host = {
        "wqT": np.ascontiguousarray(in_w[0:D].T).astype(f8),
        "wkT": np.ascontiguousarray(in_w[D:2 * D].T).astype(f8),
        "wvT": np.ascontiguousarray(in_w[2 * D:3 * D].T).astype(f8),
        "woT": np.ascontiguousarray(out_w.T).astype(f8),
        "w1T": np.ascontiguousarray(t1.T).astype(f8),
        "w2T": np.ascontiguousarray(t2.T).astype(f8),
        "ident": np.eye(128, dtype=np.float32).astype(bf),
    }

    def nz(a):
        return bool(np.any(np.asarray(a) != 0.0))

    biases = {
        "ln1_g": bool(np.any(np.asarray(inputs["ln1_g"]) != 1.0)),
        "ln1_b": nz(inputs["ln1_b"]),
        "ln2_g": bool(np.any(np.asarray(inputs["ln2_g"]) != 1.0)),
        "ln2_b": nz(inputs["ln2_b"]),
        "in_proj_b": nz(inputs["in_proj_b"]),
        "out_proj_b": nz(inputs["out_proj_b"]),
        "b1": nz(inputs["b1"]),
        "b2": nz(inputs["b2"]),
    }
    extra = {}
    if biases["ln1_g"]:
        extra["ln1_g"] = np.asarray(inputs["ln1_g"], np.float32)
    if biases["ln1_b"]:
        extra["ln1_b"] = np.asarray(inputs["ln1_b"], np.float32)
    if biases["ln2_g"]:
        extra["ln2_g"] = np.asarray(inputs["ln2_g"], np.float32)
    if biases["ln2_b"]:
        extra["ln2_b"] = np.asarray(inputs["ln2_b"], np.float32)
    if biases["in_proj_b"]:
        extra["in_b"] = np.asarray(inputs["in_proj_b"], np.float32)
    if biases["out_proj_b"]:
        extra["out_b"] = np.asarray(inputs["out_proj_b"], np.float32)
    if biases["b1"]:
        extra["b1"] = np.asarray(inputs["b1"], np.float32)
    if biases["b2"]:
        extra["b2"] = np.asarray(inputs["b2"], np.float32)

    x_flat = x.reshape(NTOK, D)
    in_maps = []
    for c in range(N_CORES):
        m = {"x_sh": np.ascontiguousarray(x_flat[c * TOK:(c + 1) * TOK])}
        m.update(host)
        m.update(extra)
        in_maps.append(m)
    return in_maps, s1, s2, biases


def get_program(s1, s2, biases, for_hw=True, sim_gelu=False):
    key = (round(s1, 12), round(s2, 12), tuple(sorted(biases.items())), for_hw,
           sim_gelu)
    if key not in _CACHE:
        nc = build_program(s1, s2, biases, sim_gelu=sim_gelu)
        if for_hw:
            nc.m = get_hw_module(nc.m)
        _CACHE[key] = nc
    return _CACHE[key]


def kernel(**inputs):
    in_maps, s1, s2, biases = _prepare(inputs)
    nc = get_program(s1, s2, biases, for_hw=True)
    res = run_bass_kernel_spmd(nc, in_maps, list(range(N_CORES)))
    out = np.concatenate([res.results[c]["out"] for c in range(N_CORES)], axis=0)
    return out.reshape(B, S, D).astype(np.float32)
